# revision 19
# baseline (speedup 1.0000x reference)
"""Trainium2 Bass kernel for nn_AlignmentLoss (8-core SPMD, no collectives).

Math: with gram = A A^T and eq[i,j] = (t_i == t_j), both symmetric,
  S1 = sum(tril(gram*eq,-1)) = sum_c (||m_c||^2 - sum_{i in c}||a_i||^2)/2
  S2 = sum(tril(eq,-1))      = (sum_c n_c^2 - N) / 2
  S3 = sum(tril(gram,-1)^2)  = (||A^T A||_F^2 - sum_i (||a_i||^2)^2) / 2
  loss = -(S1 / (S2 * sqrt(S3)))
where m_c = sum of rows with label c, n_c = count of label c.

Sharding (8 cores, SPMD, zero collectives):
  * S3: core m computes row-slice m of G = H^T H (H = bf16(A), full H is the
    moving operand on every core), then sum(G_slice^2) locally.
  * S1/S2: rows are grouped by class range [125m, 125(m+1)) so all
    same-class pairs are core-local.  The onehot matmul runs twice (H rows
    and L = bf16(A - H) rows) so m_c has f32-level accuracy; the
    cancellation-prone ||m_c||^2 - ssq_c is formed per class.
  * Each core emits 8 partial scalars; the host sums 8x8 floats and applies
    the final formula (the gather/unshard step).

Raw Bass (no TileContext): explicit semaphores; SP ring streams gcol+H
blocks (G matmuls overlap the stream), Act ring carries rr+misc for the
DVE/onehot pipeline.
"""

import numpy as np
import ml_dtypes

N, D, C = 4096, 1024, 1000
NCORES = 8
LC = C // NCORES          # 125 classes per core
CAP = 640                 # padded per-core row capacity (seed-0 max block is 547)
KT_G = N // 128           # 32 k-tiles for the G matmul
KT_R = CAP // 128         # 5 k-tiles for the onehot matmul
HB = 4                    # hf k-tiles per DMA block
NHB = KT_G // HB          # 8 hf DMA blocks
PAD_LABEL = 999.0         # outside iota range [0,128) -> onehot row of zeros

_CACHE = {}


def _build_module():
    import concourse.bass as bass
    import concourse.mybir as mybir
    from contextlib import ExitStack

    dt = mybir.dt
    AL = mybir.AluOpType
    nc = bass.Bass("TRN2", target_bir_lowering=False, debug=False)

    hf = nc.dram_tensor("hf", [N, D], dt.bfloat16, kind="ExternalInput").ap()
    gcol = nc.dram_tensor("gcol", [N, 128], dt.bfloat16, kind="ExternalInput").ap()
    rr = nc.dram_tensor("rr", [CAP, 2 * D], dt.bfloat16, kind="ExternalInput").ap()
    misc = nc.dram_tensor("misc", [128, 128 + KT_R], dt.float32,
                          kind="ExternalInput").ap()
    out = nc.dram_tensor("out", [1, 8], dt.float32, kind="ExternalOutput").ap()

    hf_t = hf.rearrange("(t p) d -> p t d", p=128)
    gcol_t = gcol.rearrange("(t p) c -> p t c", p=128)
    rr_t = rr.rearrange("(t p) d -> p t d", p=128)

    ctx = ExitStack()
    with ctx:
        sb = lambda shape, dtype, name: ctx.enter_context(
            nc.sbuf_tensor(name, shape, dtype)).ap()
        ps = lambda shape, name: ctx.enter_context(
            nc.psum_tensor(name, shape, dt.float32)).ap()

        hf_sb = sb([128, KT_G, D], dt.bfloat16, "hf_sb")
        gcol_sb = sb([128, KT_G, 128], dt.bfloat16, "gcol_sb")
        rr_sb = sb([128, KT_R, 2 * D], dt.bfloat16, "rr_sb")
        misc_sb = sb([128, 128 + KT_R], dt.float32, "misc_sb")
        rowsf = sb([128, KT_R, D], dt.float32, "rowsf")
        r_col = sb([128, KT_R], dt.float32, "r_col")
        r_hi = sb([128, KT_R], dt.bfloat16, "r_hi")
        r_hi_f = sb([128, KT_R], dt.float32, "r_hi_f")
        r_lo = sb([128, KT_R], dt.bfloat16, "r_lo")
        oh_sb = sb([128, KT_R, 128], dt.bfloat16, "oh_sb")
        ext = sb([128, KT_R, 3], dt.bfloat16, "ext")
        scr = sb([128, D], dt.float32, "scr")
        gss = sb([128, 2], dt.float32, "gss")
        m_sb = sb([128, D], dt.float32, "m_sb")
        mc_sb = sb([128, 512], dt.float32, "mc_sb")
        mc2_sb = sb([128, 512], dt.float32, "mc2_sb")
        mx_sb = sb([128, 3], dt.float32, "mx_sb")
        msq = sb([128, 1], dt.float32, "msq")
        ssq = sb([128, 1], dt.float32, "ssq")
        stats = sb([128, 8], dt.float32, "stats")
        ones_sb = sb([128, 1], dt.float32, "ones_sb")
        out_sb = sb([1, 8], dt.float32, "out_sb")

        psum_g0 = ps([128, 512], "psum_g0")
        psum_g1 = ps([128, 512], "psum_g1")
        psum_mh0 = ps([128, 512], "psum_mh0")
        psum_mh1 = ps([128, 512], "psum_mh1")
        psum_ml0 = ps([128, 512], "psum_ml0")
        psum_ml1 = ps([128, 512], "psum_ml1")
        psum_mx = ps([128, 3], "psum_mx")
        psum_st = ps([1, 8], "psum_st")

        # one semaphore per DMA: HWDGE queues complete out of order
        s_gcol = ctx.enter_context(nc.semaphore("s_gcol"))
        s_hf = [ctx.enter_context(nc.semaphore(f"s_hf{b}")) for b in range(NHB)]
        s_misc = ctx.enter_context(nc.semaphore("s_misc"))
        s_rr = ctx.enter_context(nc.semaphore("s_rr"))
        s_out = ctx.enter_context(nc.semaphore("s_out"))
        s_pe = ctx.enter_context(nc.semaphore("s_pe"))  # PE milestones
        s_v = ctx.enter_context(nc.semaphore("s_v"))    # DVE milestones
        s_c = ctx.enter_context(nc.semaphore("s_c"))    # ACT compute milestones
        s_vc = ctx.enter_context(nc.semaphore("s_vc"))  # DVE same-engine chain
        s_cc = ctx.enter_context(nc.semaphore("s_cc"))  # ACT same-engine chain

        block_cm = nc.Block()
        block = block_cm.__enter__()

        # ---------------- SP ring: gcol + hf stream + final out ------------
        @block.sync
        def _(sync):
            sync.dma_start(gcol_sb[:], gcol_t).then_inc(s_gcol, 16)
            for b in range(NHB):
                sync.dma_start(
                    hf_sb[:, HB * b:HB * (b + 1), :],
                    hf_t[:, HB * b:HB * (b + 1), :],
                ).then_inc(s_hf[b], 16)
            sync.wait_ge(s_v, 6)
            sync.dma_start(out, out_sb[:]).then_inc(s_out, 16)
            sync.wait_ge(s_out, 16)

        # ------- Act: DMA ring (misc+rr) then all square-accumulate ops -----
        AF = mybir.ActivationFunctionType

        @block.scalar
        def _(scalar):
            cc = [0]

            def link(inst):
                cc[0] += 1
                inst.then_inc(s_cc, 1)

            def sync():
                scalar.wait_ge(s_cc, cc[0])

            scalar.dma_start(misc_sb[:], misc).then_inc(s_misc, 16)
            scalar.dma_start(rr_sb[:], rr_t).then_inc(s_rr, 16)

            scalar.wait_ge(s_v, 2)   # rowsf ready
            for t in range(KT_R):
                link(nc.scalar.activation(scr[:], rowsf[:, t, :], AF.Square,
                                          accum_out=r_col[:, t:t + 1]))
                sync()
            nc.scalar.activation(scr[:, 0:KT_R], r_col[:], AF.Square,
                                 accum_out=stats[:, 1:2]).then_inc(s_c, 1)  # ->1

            scalar.wait_ge(s_pe, 1)  # G psum complete
            scalar.wait_ge(s_c, 1)   # prior scr write drained
            link(nc.scalar.activation(scr[:, 0:512], psum_g0[:], AF.Square,
                                      accum_out=gss[:, 0:1]))
            sync()
            nc.scalar.activation(scr[:, 0:512], psum_g1[:], AF.Square,
                                 accum_out=gss[:, 1:2]).then_inc(s_c, 1)    # ->2

            scalar.wait_ge(s_v, 4)   # m_sb assembled
            scalar.wait_ge(s_c, 2)   # prior scr write drained
            nc.scalar.activation(scr[:], m_sb[:], AF.Square,
                                 accum_out=msq[:]).then_inc(s_c, 1)         # ->3

        # ---------------- PE: G stream, M~, stats --------------------------
        @block.tensor
        def _(tensor):
            tensor.wait_ge(s_gcol, 16)
            mm = None
            for b in range(NHB):
                tensor.wait_ge(s_hf[b], 16)
                for kk in range(HB * b, HB * (b + 1)):
                    st, sp = (kk == 0), (kk == KT_G - 1)
                    nc.tensor.matmul(psum_g0[:], gcol_sb[:, kk, :],
                                     hf_sb[:, kk, 0:512], start=st, stop=sp)
                    mm = nc.tensor.matmul(psum_g1[:], gcol_sb[:, kk, :],
                                          hf_sb[:, kk, 512:1024], start=st, stop=sp)
            mm.then_inc(s_pe, 1)                                           # G done

            tensor.wait_ge(s_rr, 16)  # rr resident
            tensor.wait_ge(s_v, 1)    # onehots ready
            for t in range(KT_R):
                st, sp = (t == 0), (t == KT_R - 1)
                oh_t = oh_sb[:, t, :]
                nc.tensor.matmul(psum_mh0[:], oh_t, rr_sb[:, t, 0:512],
                                 start=st, stop=sp)
                nc.tensor.matmul(psum_mh1[:], oh_t, rr_sb[:, t, 512:1024],
                                 start=st, stop=sp)
                nc.tensor.matmul(psum_ml0[:], oh_t, rr_sb[:, t, D:D + 512],
                                 start=st, stop=sp)
                mm = nc.tensor.matmul(psum_ml1[:], oh_t, rr_sb[:, t, D + 512:2 * D],
                                      start=st, stop=sp)
            mm.then_inc(s_pe, 1)                                           # ->2

            tensor.wait_ge(s_v, 3)    # ext cols ready
            for t in range(KT_R):
                mm = nc.tensor.matmul(psum_mx[:], oh_sb[:, t, :], ext[:, t, :],
                                      start=(t == 0), stop=(t == KT_R - 1))
            mm.then_inc(s_pe, 1)                                           # ->3

            tensor.wait_ge(s_v, 5)    # stats cols written
            nc.tensor.matmul(psum_st[:], ones_sb[:], stats[:],
                             start=True, stop=True).then_inc(s_pe, 1)      # ->4

        # ---------------- DVE: adds / copies / onehot -----------------------
        @block.vector
        def _(vector):
            vc = [0]

            def link(inst):
                vc[0] += 1
                inst.then_inc(s_vc, 1)

            def sync():
                vector.wait_ge(s_vc, vc[0])

            nc.vector.memset(stats[:], 0.0)
            nc.vector.memset(ones_sb[:], 1.0)

            vector.wait_ge(s_misc, 16)
            for t in range(KT_R):
                inst = nc.vector.tensor_scalar(
                    out=oh_sb[:, t, :], in0=misc_sb[:, 0:128],
                    scalar1=misc_sb[:, 128 + t:129 + t], scalar2=None,
                    op0=AL.is_equal,
                )
            inst.then_inc(s_v, 1)                                          # ->1

            vector.wait_ge(s_rr, 16)
            for t in range(KT_R):
                inst = nc.vector.tensor_add(rowsf[:, t, :], rr_sb[:, t, 0:D],
                                            rr_sb[:, t, D:2 * D])
            inst.then_inc(s_v, 1)                                          # ->2

            vector.wait_ge(s_c, 1)    # r_col ready
            link(nc.vector.tensor_copy(r_hi[:], r_col[:]))
            sync()
            link(nc.vector.tensor_copy(r_hi_f[:], r_hi[:]))
            sync()
            link(nc.vector.tensor_sub(r_lo[:], r_col[:], r_hi_f[:]))
            nc.vector.memset(ext[:, :, 0:1], 1.0)
            sync()
            for t in range(KT_R):
                nc.vector.tensor_copy(ext[:, t, 1:2], r_hi[:, t:t + 1])
                inst = nc.vector.tensor_copy(ext[:, t, 2:3], r_lo[:, t:t + 1])
            inst.then_inc(s_v, 1)                                          # ->3

            vector.wait_ge(s_pe, 3)   # M~ + ext psum complete
            link(nc.vector.tensor_copy(mc_sb[:], psum_mh0[:]))
            link(nc.vector.tensor_copy(mc2_sb[:], psum_mh1[:]))
            sync()
            nc.vector.tensor_add(m_sb[:, 0:512], mc_sb[:], psum_ml0[:])
            nc.vector.tensor_add(m_sb[:, 512:1024], mc2_sb[:],
                                 psum_ml1[:]).then_inc(s_v, 1)             # ->4 (ACT msq)

            vector.wait_ge(s_c, 2)    # gss ready
            nc.vector.tensor_add(stats[:, 0:1], gss[:, 0:1], gss[:, 1:2])

            link(nc.vector.tensor_copy(mx_sb[:], psum_mx[:]))
            sync()
            link(nc.vector.tensor_add(ssq[:], mx_sb[:, 1:2], mx_sb[:, 2:3]))
            nc.vector.tensor_mul(stats[:, 3:4], mx_sb[:, 0:1], mx_sb[:, 0:1])
            nc.vector.tensor_copy(stats[:, 4:5], mx_sb[:, 0:1])
            vector.wait_ge(s_c, 3)    # msq ready
            sync()
            nc.vector.tensor_sub(stats[:, 2:3], msq[:], ssq[:]).then_inc(s_v, 1)  # ->5

            vector.wait_ge(s_pe, 4)   # stats matmul done
            nc.vector.tensor_copy(out_sb[:], psum_st[:]).then_inc(s_v, 1)  # ->6

        # -------- finalizer: return all sems to 0 for safe re-execution -----
        block_cm.__exit__(None, None, None)
        block2_cm = nc.Block(name="finalize")
        block2 = block2_cm.__enter__()

        @block2.gpsimd
        def _(g):
            for sem in [s_gcol, *s_hf, s_misc, s_rr, s_out, s_pe, s_v, s_c,
                        s_vc, s_cc]:
                g.sem_clear(sem)

        block2_cm.__exit__(None, None, None)

    return nc


def _prepare_inputs(output, target):
    A = np.ascontiguousarray(np.asarray(output, dtype=np.float32))
    t = np.asarray(target).astype(np.int64)
    H = A.astype(ml_dtypes.bfloat16)
    L = (A - H.astype(np.float32)).astype(ml_dtypes.bfloat16)

    group = t // LC
    in_maps = []
    for m in range(NCORES):
        sel = np.nonzero(group == m)[0]
        assert len(sel) <= CAP, f"core {m} has {len(sel)} rows > CAP={CAP}"
        rr = np.zeros((CAP, 2 * D), dtype=ml_dtypes.bfloat16)
        lbl = np.full((CAP,), PAD_LABEL, dtype=np.float32)
        rr[: len(sel), :D] = H[sel]
        rr[: len(sel), D:] = L[sel]
        lbl[: len(sel)] = (t[sel] - LC * m).astype(np.float32)
        misc = np.zeros((128, 128 + KT_R), dtype=np.float32)
        misc[:, :128] = np.arange(128, dtype=np.float32)[None, :]
        misc[:, 128:] = lbl.reshape(KT_R, 128).T
        in_maps.append(
            {
                "hf": H,
                "gcol": np.ascontiguousarray(H[:, 128 * m:128 * (m + 1)]),
                "rr": rr,
                "misc": misc,
            }
        )
    return in_maps


def _combine(partials):
    P = np.stack([np.asarray(p, dtype=np.float64).reshape(8) for p in partials])
    tot = P.sum(axis=0)
    gss, r2s, crs, n2s = tot[0], tot[1], tot[2], tot[3]
    S3 = (gss - r2s) / 2.0
    S1 = crs / 2.0
    S2 = (n2s - N) / 2.0
    loss = -(S1 / (S2 * np.sqrt(S3)))
    return np.float32(loss)


def kernel(output, target):
    from concourse.bass_utils import run_bass_kernel_spmd

    if "nc" not in _CACHE:
        _CACHE["nc"] = _build_module()
    nc = _CACHE["nc"]
    in_maps = _prepare_inputs(output, target)
    res = run_bass_kernel_spmd(nc, in_maps, core_ids=list(range(NCORES)))
    return _combine([r["out"] for r in res.results])
